# revision 13
# baseline (speedup 1.0000x reference)
"""Chebyshev-distance conv2d (p=inf "Conv2d") Trainium2 kernel.

Problem: y[b,o,ho,wo] = max_k |patch[b,k,ho,wo] - wf[o,k]|,
  B=8, C=32, O=64, H=W=48, 3x3 kernel, stride 1, pad 1, K = C*9 = 288.

Strategy (8 NeuronCores, data-parallel over batch, 1 image per core):
  - Partition dim (128) = 64 output channels x 2 spatial halves
    (rows 0..23 on partitions 0..63, rows 24..47 on partitions 64..127).
  - TensorE broadcasts each padded input-channel slab (26 rows x 50 cols,
    one slab per half) to all 128 partitions with a ones-matmul
    (contraction dim 2) into PSUM, double buffered.
  - ScalarE computes |w[o,k] - x| via activation(Abs, scale=-1,
    bias=w[o,k] per partition) reading tap-shifted views of the PSUM slab.
  - VectorE max-accumulates into the fp32 accumulator.
"""

import sys

if "/opt/trn_rl_repo" not in sys.path:
    sys.path.insert(0, "/opt/trn_rl_repo")

import numpy as np

import concourse.bass as bass
import concourse.bacc as bacc
import concourse.mybir as mybir
from concourse.tile import TileContext
from concourse.bass_utils import run_bass_kernel_spmd

B, C, O, H, W = 8, 32, 64, 48, 48
KS, PAD = 3, 1
HO, WO = 48, 48
K = C * KS * KS          # 288
NHALF = HO // 2          # 24 output rows per half
SLAB_R, SLAB_C = NHALF + 2, W + 2   # 26 x 50 padded slab per half
SLAB = SLAB_R * SLAB_C   # 1300
CGROUPS, CPG = 4, 8      # channel groups of 8 -> staging partitions = 2*4=8

F32 = mybir.dt.float32


def build_nc():
    nc = bacc.Bacc(trn_type="TRN2")

    x_slab = nc.declare_dram_parameter(
        "x_slab", [2, C, SLAB_R, SLAB_C], F32, isOutput=False
    )
    wbias = nc.declare_dram_parameter("wbias", [128, K], F32, isOutput=False)
    ones2 = nc.declare_dram_parameter("ones2", [2, 128], F32, isOutput=False)
    out = nc.declare_dram_parameter("out", [128, NHALF, WO], F32, isOutput=True)

    with TileContext(nc) as tc:
        with (
            tc.tile_pool(name="const", bufs=1) as cpool,
            tc.tile_pool(name="work", bufs=3) as wpool,
            tc.tile_pool(name="psum", bufs=1, space="PSUM") as ppool,
        ):
            wb = cpool.tile([128, K], F32)
            ones = cpool.tile([2, 128], F32)
            acc = cpool.tile([128, NHALF, WO], F32)
            stage = cpool.tile([2, C, SLAB_R, SLAB_C], F32)
            # Two persistent PSUM slabs, manually alternated per channel —
            # rotating pool slots would put >1 sem wait on the matmuls
            # (walrus allows only one on LDWEIGHTS).
            slab_a = ppool.tile([128, SLAB_R, SLAB_C], F32, tag="slab_a")
            slab_b = ppool.tile([128, SLAB_R, SLAB_C], F32, tag="slab_b")
            slabs = [slab_a, slab_b]

            # Single-queue SWDGE loads so consumers wait on one DMA sem.
            nc.gpsimd.dma_start(stage[:], x_slab[:])
            nc.gpsimd.dma_start(wb[:], wbias[:])
            nc.gpsimd.dma_start(ones[:], ones2[:])
            nc.vector.memset(acc[:], 0.0)

            # Dummy 1-column matmul: absorbs the `ones` DMA wait on PE so
            # real matmuls carry at most one sem wait (LDWEIGHTS limit).
            slab0_f = slabs[0].rearrange("p r c -> p (r c)")
            nc.tensor.matmul(
                slab0_f[:, 0:1], ones[:], ones[:, 0:1], start=True, stop=True
            )

            for c in range(C):
                slab = slabs[c % 2]
                slab_f = slab.rearrange("p r c -> p (r c)")
                rhs = stage[:, c].rearrange("p r c -> p (r c)")
                # Broadcast both halves of channel c to the 128 partitions.
                for off in range(0, SLAB, 512):
                    sz = min(512, SLAB - off)
                    nc.tensor.matmul(
                        slab_f[:, off : off + sz],
                        ones[:],
                        rhs[:, off : off + sz],
                        start=True,
                        stop=True,
                    )
                for tap in range(KS * KS):
                    kh, kw = tap // KS, tap % KS
                    k = c * (KS * KS) + tap
                    tmp = wpool.tile([128, NHALF, WO], F32, tag="tmp")
                    # tmp = |w[o,k] - x_tap|
                    nc.scalar.activation(
                        tmp[:],
                        slab[:, kh : kh + NHALF, kw : kw + WO],
                        mybir.ActivationFunctionType.Abs,
                        bias=wb[:, k : k + 1],
                        scale=-1.0,
                    )
                    nc.vector.tensor_tensor(
                        acc[:], acc[:], tmp[:], op=mybir.AluOpType.max
                    )

            nc.sync.dma_start(out[:], acc[:])

    nc.compile()
    return nc


_NC_CACHE = {}


def _get_nc():
    if "nc" not in _NC_CACHE:
        _NC_CACHE["nc"] = build_nc()
    return _NC_CACHE["nc"]


def make_in_maps(inputs: np.ndarray, weights: np.ndarray):
    x = np.asarray(inputs, dtype=np.float32)
    w = np.asarray(weights, dtype=np.float32)
    assert x.shape == (B, C, H, W) and w.shape == (O, C, KS, KS)

    xp = np.zeros((B, C, H + 2 * PAD, W + 2 * PAD), np.float32)
    xp[:, :, PAD : PAD + H, PAD : PAD + W] = x
    half_a = xp[:, :, 0:SLAB_R, :]                    # (B, C, 26, 50)
    half_b = xp[:, :, NHALF : NHALF + SLAB_R, :]      # (B, C, 26, 50)
    halves = np.stack([half_a, half_b], axis=2)       # (B, C, 2, 26, 50)
    stage = halves.transpose(0, 2, 1, 3, 4)           # (B, 2, C, 26, 50)

    wf = w.reshape(O, K)
    wb = np.ascontiguousarray(np.tile(wf, (2, 1)))    # (128, K)
    ones2 = np.zeros((2, 128), np.float32)
    ones2[0, :64] = 1.0
    ones2[1, 64:] = 1.0

    return [
        {
            "x_slab": np.ascontiguousarray(stage[b]),
            "wbias": wb,
            "ones2": ones2,
        }
        for b in range(B)
    ]


def assemble_output(results):
    y = np.empty((B, O, HO, WO), np.float32)
    for b in range(B):
        o = results[b]["out"]
        y[b, :, :NHALF, :] = o[0:64]
        y[b, :, NHALF:, :] = o[64:128]
    return y


def launch(inputs: np.ndarray, weights: np.ndarray, trace: bool = False):
    """Run on 8 NeuronCores; returns (y, BassKernelResults)."""
    in_maps = make_in_maps(inputs, weights)
    res = run_bass_kernel_spmd(
        _get_nc(), in_maps, list(range(B)), trace=trace
    )
    return assemble_output(res.results), res


def kernel(inputs: np.ndarray, weights: np.ndarray) -> np.ndarray:
    y, _ = launch(inputs, weights, trace=False)
    return y


# revision 17
# speedup vs baseline: 1.1820x; 1.1820x over previous
"""Chebyshev-distance conv2d (p=inf "Conv2d") Trainium2 kernel.

Problem: y[b,o,ho,wo] = max_k |patch[b,k,ho,wo] - wf[o,k]|,
  B=8, C=32, O=64, H=W=48, 3x3 kernel, stride 1, pad 1, K = C*9 = 288.

Strategy (8 NeuronCores, data-parallel over batch, 1 image per core):
  - Partition dim (128) = 64 output channels x 2 spatial halves
    (rows 0..23 on partitions 0..63, rows 24..47 on partitions 64..127).
  - TensorE broadcasts each padded input-channel slab (26 rows x 50 cols,
    one slab per half) to all 128 partitions with a ones-matmul
    (contraction dim 2) into PSUM, double buffered.
  - ScalarE computes |w[o,k] - x| via activation(Abs, scale=-1,
    bias=w[o,k] per partition) reading tap-shifted views of the PSUM slab.
  - VectorE max-accumulates into the fp32 accumulator.
"""

import sys

if "/opt/trn_rl_repo" not in sys.path:
    sys.path.insert(0, "/opt/trn_rl_repo")

import numpy as np

import concourse.bass as bass
import concourse.bacc as bacc
import concourse.mybir as mybir
from concourse.tile import TileContext
from concourse.bass_utils import run_bass_kernel_spmd

B, C, O, H, W = 8, 32, 64, 48, 48
KS, PAD = 3, 1
HO, WO = 48, 48
K = C * KS * KS          # 288
NHALF = HO // 2          # 24 output rows per half
SLAB_R, SLAB_C = NHALF + 2, W + 2   # 26 x 50 padded slab per half
SLAB = SLAB_R * SLAB_C   # 1300
CGROUPS, CPG = 4, 8      # channel groups of 8 -> staging partitions = 2*4=8

F32 = mybir.dt.float32
BF16 = mybir.dt.bfloat16


def build_nc():
    nc = bacc.Bacc(trn_type="TRN2")

    x_slab = nc.declare_dram_parameter(
        "x_slab", [2, C, SLAB_R, SLAB_C], F32, isOutput=False
    )
    wbias = nc.declare_dram_parameter("wbias", [128, K], F32, isOutput=False)
    ones2 = nc.declare_dram_parameter("ones2", [2, 128], F32, isOutput=False)
    out = nc.declare_dram_parameter("out", [128, NHALF, WO], F32, isOutput=True)

    with TileContext(nc) as tc:
        with (
            tc.tile_pool(name="const", bufs=1) as cpool,
            tc.tile_pool(name="work", bufs=3) as wpool,
            tc.tile_pool(name="psum", bufs=1, space="PSUM") as ppool,
        ):
            wb = cpool.tile([128, K], F32)
            ones = cpool.tile([2, 128], F32)
            acc = cpool.tile([128, NHALF, WO], BF16)
            acc32 = cpool.tile([128, NHALF, WO], F32)
            stage = cpool.tile([2, C, SLAB_R, SLAB_C], F32)
            # Two persistent PSUM slabs, manually alternated per channel —
            # rotating pool slots would put >1 sem wait on the matmuls
            # (walrus allows only one on LDWEIGHTS).
            slab_a = ppool.tile([128, SLAB_R, SLAB_C], F32, tag="slab_a")
            slab_b = ppool.tile([128, SLAB_R, SLAB_C], F32, tag="slab_b")
            slabs = [slab_a, slab_b]

            # Single-queue SWDGE loads so consumers wait on one DMA sem.
            nc.gpsimd.dma_start(stage[:], x_slab[:])
            nc.gpsimd.dma_start(wb[:], wbias[:])
            nc.gpsimd.dma_start(ones[:], ones2[:])
            nc.vector.memset(acc[:], 0.0)

            # Dummy 1-column matmul: absorbs the `ones` DMA wait on PE so
            # real matmuls carry at most one sem wait (LDWEIGHTS limit).
            slab0_f = slabs[0].rearrange("p r c -> p (r c)")
            nc.tensor.matmul(
                slab0_f[:, 0:1], ones[:], ones[:, 0:1], start=True, stop=True
            )

            for c in range(C):
                slab = slabs[c % 2]
                slab_f = slab.rearrange("p r c -> p (r c)")
                rhs = stage[:, c].rearrange("p r c -> p (r c)")
                # Broadcast both halves of channel c to the 128 partitions.
                for off in range(0, SLAB, 512):
                    sz = min(512, SLAB - off)
                    nc.tensor.matmul(
                        slab_f[:, off : off + sz],
                        ones[:],
                        rhs[:, off : off + sz],
                        start=True,
                        stop=True,
                    )
                for tap in range(KS * KS):
                    kh, kw = tap // KS, tap % KS
                    k = c * (KS * KS) + tap
                    tmp = wpool.tile([128, NHALF, WO], BF16, tag="tmp")
                    # tmp = |w[o,k] - x_tap|
                    nc.scalar.activation(
                        tmp[:],
                        slab[:, kh : kh + NHALF, kw : kw + WO],
                        mybir.ActivationFunctionType.Abs,
                        bias=wb[:, k : k + 1],
                        scale=-1.0,
                    )
                    nc.vector.tensor_tensor(
                        acc[:], acc[:], tmp[:], op=mybir.AluOpType.max
                    )

            nc.scalar.copy(acc32[:], acc[:])
            nc.sync.dma_start(out[:], acc32[:])

    nc.compile()
    return nc


_NC_CACHE = {}


def _get_nc():
    if "nc" not in _NC_CACHE:
        _NC_CACHE["nc"] = build_nc()
    return _NC_CACHE["nc"]


def make_in_maps(inputs: np.ndarray, weights: np.ndarray):
    x = np.asarray(inputs, dtype=np.float32)
    w = np.asarray(weights, dtype=np.float32)
    assert x.shape == (B, C, H, W) and w.shape == (O, C, KS, KS)

    xp = np.zeros((B, C, H + 2 * PAD, W + 2 * PAD), np.float32)
    xp[:, :, PAD : PAD + H, PAD : PAD + W] = x
    half_a = xp[:, :, 0:SLAB_R, :]                    # (B, C, 26, 50)
    half_b = xp[:, :, NHALF : NHALF + SLAB_R, :]      # (B, C, 26, 50)
    halves = np.stack([half_a, half_b], axis=2)       # (B, C, 2, 26, 50)
    stage = halves.transpose(0, 2, 1, 3, 4)           # (B, 2, C, 26, 50)

    wf = w.reshape(O, K)
    wb = np.ascontiguousarray(np.tile(wf, (2, 1)))    # (128, K)
    ones2 = np.zeros((2, 128), np.float32)
    ones2[0, :64] = 1.0
    ones2[1, 64:] = 1.0

    return [
        {
            "x_slab": np.ascontiguousarray(stage[b]),
            "wbias": wb,
            "ones2": ones2,
        }
        for b in range(B)
    ]


def assemble_output(results):
    y = np.empty((B, O, HO, WO), np.float32)
    for b in range(B):
        o = results[b]["out"]
        y[b, :, :NHALF, :] = o[0:64]
        y[b, :, NHALF:, :] = o[64:128]
    return y


def launch(inputs: np.ndarray, weights: np.ndarray, trace: bool = False):
    """Run on 8 NeuronCores; returns (y, BassKernelResults)."""
    in_maps = make_in_maps(inputs, weights)
    res = run_bass_kernel_spmd(
        _get_nc(), in_maps, list(range(B)), trace=trace
    )
    return assemble_output(res.results), res


def kernel(inputs: np.ndarray, weights: np.ndarray) -> np.ndarray:
    y, _ = launch(inputs, weights, trace=False)
    return y
